# revision 13
# baseline (speedup 1.0000x reference)
"""KMeans assignment (vq_codebook) Trainium2 kernel.

argmin_k ||x_b - c_k||^2 for X[65536,1024], C[1024,1024], 8 NeuronCores,
data-parallel over the batch (8192 rows/core), centroids replicated.

Math: argmin_k d2 = argmax_k (X@C^T - ||c||^2/2); row term ||x||^2 dropped.
The cross term is a single float32r matmul pass: the PE array truncates
fp32 inputs to fp22 (13 mantissa bits), which keeps the argmax intact to
~25/65536 near-tie flips on hardware, and with a moving free dim of 512
the fp32r matmul streams at the full one-row/cycle bf16 rate. The
||c||^2/2 bias is precomputed on the host, broadcast to all partitions,
and subtracted on the Vector engine; argmax uses the DVE max/max_index
ops.

The kernel is paced end-to-end by the DVE epilogue (sub+max+max_index =
3446ns/tile vs the PE's 3413ns/tile), so total = (time tile 0's psum is
ready) + 64 x 3446ns + drain. Hence the geometric X block ramp: small
blocks first so tile 0 isn't gated on a full 4MB block transfer
serialized behind the 4MB C transfer, and warmup matmuls to hold the PE
p-state up while the first DMAs land.
"""
import numpy as np
import concourse.bacc as bacc
import concourse.mybir as mybir
from concourse.tile import TileContext
from concourse.bass_utils import run_bass_kernel_spmd

B, F, K = 65536, 1024, 1024
NCORES = 8
BL = B // NCORES          # rows per core
P = 128
FCH = F // P              # 8 contraction chunks
NH = 512                  # psum half (max fp32 moving operand / bank)
# X DMA block sizes (columns): small blocks first so tile 0 isn't gated on a
# full 4MB block transfer behind the 4MB C transfer — the kernel is paced by
# the DVE epilogue from tile 0 onward, so startup latency is pure total time.
BLOCKS = [256, 256, 512] + [1024] * 7
NWARM = 8                 # p-state warmup matmuls
DT = mybir.dt.float32r

_NC_CACHE = {}


def _build(bl):
    assert sum(BLOCKS) == bl
    nb = bl // P
    nc = bacc.Bacc("TRN2", target_bir_lowering=False)
    x = nc.dram_tensor("x", [F, bl], DT, kind="ExternalInput")
    c = nc.dram_tensor("c", [F, K], DT, kind="ExternalInput")
    c2h = nc.dram_tensor("c2h", [K], mybir.dt.float32, kind="ExternalInput")
    out = nc.dram_tensor("out", [nb, P, 1], mybir.dt.uint32, kind="ExternalOutput")

    with TileContext(nc) as tc:
        with (
            tc.tile_pool(name="cres", bufs=1) as cres,
            tc.tile_pool(name="xp", bufs=2) as xp,
            tc.tile_pool(name="work", bufs=3) as work,
            tc.tile_pool(name="psp", bufs=4, space="PSUM") as psp,
        ):
            # Warmup: dummy matmuls into a discarded psum tile eat the PE
            # low/mid p-state ramp while the first input DMAs are in flight,
            # so real matmuls start at full clock.
            dumt = work.tile([P, NH], DT, tag="warm")
            nc.vector.memzero(dumt)
            dps = psp.tile([P, K], mybir.dt.float32, tag="ps")
            for w in range(NWARM):
                nc.tensor.matmul(dps[:, 0:NH], dumt[:, 0:P], dumt,
                                 start=(w == 0), stop=(w == NWARM - 1))

            def load_chunk(c0, ncols, f):
                t = xp.tile([P, ncols], DT, tag=f"x{f}", name=f"xt{f}")
                nc.sync.dma_start(t, x[f * P:(f + 1) * P, c0:c0 + ncols])
                return t

            # Interleave resident-C chunk loads with block-0 X chunk loads so
            # the f-th matmul of tile 0 only waits on the f-th pair; the c2
            # broadcast (needed by the first sub, several tiles later) rides
            # along early.
            c_sb = []
            blk0 = []
            c2b = cres.tile([P, K], mybir.dt.float32)
            for f in range(FCH):
                t_c = cres.tile([P, K], DT, tag=f"c{f}")
                nc.sync.dma_start(t_c, c[f * P:(f + 1) * P, :])
                c_sb.append(t_c)
                blk0.append(load_chunk(0, BLOCKS[0], f))
                if f == 0:
                    nc.sync.dma_start(c2b, c2h[None, :].to_broadcast([P, K]))

            t = 0
            c0 = 0
            for bi, ncols in enumerate(BLOCKS):
                x_t = blk0 if bi == 0 else [load_chunk(c0, ncols, f)
                                            for f in range(FCH)]
                for i in range(ncols // P):
                    ps = psp.tile([P, K], mybir.dt.float32, tag="ps")
                    for f in range(FCH):
                        first = f == 0
                        last = f == FCH - 1
                        w = x_t[f][:, i * P:(i + 1) * P]
                        nc.tensor.matmul(ps[:, 0:NH], w, c_sb[f][:, 0:NH],
                                         start=first, stop=last)
                        nc.tensor.matmul(ps[:, NH:K], w, c_sb[f][:, NH:K],
                                         start=first, stop=last)
                    a_sb = work.tile([P, K], mybir.dt.float32, tag="a")
                    nc.vector.tensor_sub(a_sb, ps, c2b)
                    mx = work.tile([P, 8], mybir.dt.float32, tag="mx")
                    nc.vector.max(out=mx, in_=a_sb)
                    ix = work.tile([P, 8], mybir.dt.uint32, tag="ix")
                    nc.vector.max_index(ix, mx, a_sb)
                    nc.sync.dma_start(out[t], ix[:, 0:1])
                    t += 1
                c0 += ncols
    nc.finalize()
    return nc


def _get_nc(bl):
    if bl not in _NC_CACHE:
        _NC_CACHE[bl] = _build(bl)
    return _NC_CACHE[bl]


def _prep_in_maps(X, C):
    X = np.ascontiguousarray(np.asarray(X, dtype=np.float32))
    C = np.ascontiguousarray(np.asarray(C, dtype=np.float32))
    assert X.shape == (B, F) and C.shape == (K, F)
    xt = np.ascontiguousarray(X.T)
    ct = np.ascontiguousarray(C.T)
    c2h = (0.5 * np.sum(C.astype(np.float64) ** 2, axis=1)).astype(np.float32)
    in_maps = []
    for cid in range(NCORES):
        sl = slice(cid * BL, (cid + 1) * BL)
        in_maps.append({
            "x": np.ascontiguousarray(xt[:, sl]),
            "c": ct,
            "c2h": c2h,
        })
    return in_maps


def kernel(X, centroids):
    nc = _get_nc(BL)
    in_maps = _prep_in_maps(X, centroids)
    res = run_bass_kernel_spmd(nc, in_maps, core_ids=list(range(NCORES)))
    out = np.concatenate([r["out"].reshape(-1) for r in res.results])
    return out.astype(np.int32)


# revision 21
# speedup vs baseline: 1.0215x; 1.0215x over previous
"""KMeans assignment (vq_codebook) Trainium2 kernel.

argmin_k ||x_b - c_k||^2 for X[65536,1024], C[1024,1024], 8 NeuronCores,
data-parallel over the batch (8192 rows/core), centroids replicated.

Math: argmin_k d2 = argmax_k (X@C^T - ||c||^2/2); row term ||x||^2 dropped.
The cross term is a single float32r matmul pass: the PE array truncates
fp32 inputs to fp22 (13 mantissa bits), which keeps the argmax intact to
~25/65536 near-tie flips on hardware, and with a moving free dim of 512
the fp32r matmul streams at the full one-row/cycle bf16 rate. The
||c||^2/2 bias is precomputed on the host, broadcast to all partitions,
and subtracted on the Vector engine; argmax uses the DVE max/max_index
ops.

The kernel is paced end-to-end by the DVE epilogue (sub+max+max_index =
3446ns/tile vs the PE's 3413ns/tile), so total = (time tile 0's psum is
ready) + 64 x 3446ns + drain. Hence the geometric X block ramp: small
blocks first so tile 0 isn't gated on a full 4MB block transfer
serialized behind the 4MB C transfer, and warmup matmuls to hold the PE
p-state up while the first DMAs land.
"""
import numpy as np
import concourse.bacc as bacc
import concourse.mybir as mybir
from concourse.tile import TileContext
from concourse.bass_utils import run_bass_kernel_spmd

B, F, K = 65536, 1024, 1024
NCORES = 8
BL = B // NCORES          # rows per core
P = 128
FCH = F // P              # 8 contraction chunks
NH = 512                  # psum half (max fp32 moving operand / bank)
# X DMA block sizes (columns): small blocks first so tile 0 isn't gated on a
# full 4MB block transfer behind the 4MB C transfer — the kernel is paced by
# the DVE epilogue from tile 0 onward, so startup latency is pure total time.
BLOCKS = [256, 256, 512] + [1024] * 7
NWARM = 8                 # p-state warmup matmuls
DT = mybir.dt.float32r

_NC_CACHE = {}


def _build(bl):
    assert sum(BLOCKS) == bl
    nb = bl // P
    nc = bacc.Bacc("TRN2", target_bir_lowering=False)
    x = nc.dram_tensor("x", [F, bl], DT, kind="ExternalInput")
    c = nc.dram_tensor("c", [F, K], DT, kind="ExternalInput")
    c2h = nc.dram_tensor("c2h", [K], mybir.dt.float32, kind="ExternalInput")
    out = nc.dram_tensor("out", [nb, P, 1], mybir.dt.uint32, kind="ExternalOutput")
    om = nc.dram_tensor("om", [nb, P, NH], mybir.dt.float32,
                        kind="ExternalOutput")

    with TileContext(nc) as tc:
        with (
            tc.tile_pool(name="cres", bufs=1) as cres,
            tc.tile_pool(name="xp", bufs=2) as xp,
            tc.tile_pool(name="work", bufs=3) as work,
            tc.tile_pool(name="psp", bufs=4, space="PSUM") as psp,
        ):
            # Warmup: dummy matmuls into a discarded psum tile eat the PE
            # low/mid p-state ramp while the first input DMAs are in flight,
            # so real matmuls start at full clock.
            dumt = work.tile([P, NH], DT, tag="warm")
            nc.vector.memzero(dumt)
            dps = psp.tile([P, K], mybir.dt.float32, tag="ps")
            for w in range(NWARM):
                nc.tensor.matmul(dps[:, 0:NH], dumt[:, 0:P], dumt,
                                 start=(w == 0), stop=(w == NWARM - 1))

            def load_chunk(c0, ncols, f):
                t = xp.tile([P, ncols], DT, tag=f"x{f}", name=f"xt{f}")
                nc.sync.dma_start(t, x[f * P:(f + 1) * P, c0:c0 + ncols])
                return t

            # Interleave resident-C chunk loads with block-0 X chunk loads so
            # the f-th matmul of tile 0 only waits on the f-th pair; the c2
            # broadcast (needed by the first sub, several tiles later) rides
            # along early.
            c_sb = []
            blk0 = []
            c2b = cres.tile([P, K], mybir.dt.float32)
            for f in range(FCH):
                t_c = cres.tile([P, K], DT, tag=f"c{f}")
                nc.sync.dma_start(t_c, c[f * P:(f + 1) * P, :])
                c_sb.append(t_c)
                blk0.append(load_chunk(0, BLOCKS[0], f))
            # c2b rides the serial DMA stream AFTER block 0 — it's not needed
            # until the first sub (~20us), and placing it earlier delays the
            # tile-0 inputs that gate the whole pipeline.
            nc.sync.dma_start(c2b, c2h[None, :].to_broadcast([P, K]))

            t = 0
            c0 = 0
            for bi, ncols in enumerate(BLOCKS):
                x_t = blk0 if bi == 0 else [load_chunk(c0, ncols, f)
                                            for f in range(FCH)]
                for i in range(ncols // P):
                    ps = psp.tile([P, K], mybir.dt.float32, tag="ps")
                    for f in range(FCH):
                        first = f == 0
                        last = f == FCH - 1
                        w = x_t[f][:, i * P:(i + 1) * P]
                        nc.tensor.matmul(ps[:, 0:NH], w, c_sb[f][:, 0:NH],
                                         start=first, stop=last)
                        nc.tensor.matmul(ps[:, NH:K], w, c_sb[f][:, NH:K],
                                         start=first, stop=last)
                    a_sb = work.tile([P, K], mybir.dt.float32, tag="a")
                    nc.vector.tensor_sub(a_sb, ps, c2b)
                    # DVE folds the 1024 scores to 512 so max/max_index scan
                    # half the data (DVE 2.97us/tile, under the PE's 3.41 —
                    # the PE becomes the pacer). The which-half bit needed to
                    # decode the folded index is computed on the otherwise
                    # idle Pool engine and shipped to the host, so nothing
                    # feeds back into the DVE chain.
                    m = work.tile([P, NH], mybir.dt.float32, tag="m")
                    nc.vector.tensor_max(m, a_sb[:, 0:NH], a_sb[:, NH:K])
                    d = work.tile([P, NH], mybir.dt.float32, tag="d")
                    nc.gpsimd.tensor_sub(d, a_sb[:, 0:NH], a_sb[:, NH:K])
                    mx = work.tile([P, 8], mybir.dt.float32, tag="mx")
                    nc.vector.max(out=mx, in_=m)
                    ix = work.tile([P, 8], mybir.dt.uint32, tag="ix")
                    nc.vector.max_index(ix, mx, m)
                    nc.sync.dma_start(out[t], ix[:, 0:1])
                    nc.scalar.dma_start(om[t], d)
                    t += 1
                c0 += ncols
    nc.finalize()
    return nc


def _get_nc(bl):
    if bl not in _NC_CACHE:
        _NC_CACHE[bl] = _build(bl)
    return _NC_CACHE[bl]


def _prep_in_maps(X, C):
    X = np.ascontiguousarray(np.asarray(X, dtype=np.float32))
    C = np.ascontiguousarray(np.asarray(C, dtype=np.float32))
    assert X.shape == (B, F) and C.shape == (K, F)
    xt = np.ascontiguousarray(X.T)
    ct = np.ascontiguousarray(C.T)
    c2h = (0.5 * np.sum(C.astype(np.float64) ** 2, axis=1)).astype(np.float32)
    in_maps = []
    for cid in range(NCORES):
        sl = slice(cid * BL, (cid + 1) * BL)
        in_maps.append({
            "x": np.ascontiguousarray(xt[:, sl]),
            "c": ct,
            "c2h": c2h,
        })
    return in_maps


def kernel(X, centroids):
    nc = _get_nc(BL)
    in_maps = _prep_in_maps(X, centroids)
    res = run_bass_kernel_spmd(nc, in_maps, core_ids=list(range(NCORES)))
    outs = []
    for r in res.results:
        j = r["out"].reshape(-1).astype(np.int64)          # folded index
        dm = r["om"].reshape(-1, NH)                       # a_lo - a_hi rows
        d = np.take_along_axis(dm, j[:, None], axis=1).reshape(-1)
        outs.append(np.where(d >= 0, j, j + NH))           # decode true k
    return np.concatenate(outs).astype(np.int32)


# revision 22
# speedup vs baseline: 1.0240x; 1.0024x over previous
"""KMeans assignment (vq_codebook) Trainium2 kernel.

argmin_k ||x_b - c_k||^2 for X[65536,1024], C[1024,1024], 8 NeuronCores,
data-parallel over the batch (8192 rows/core), centroids replicated.

Math: argmin_k d2 = argmax_k (X@C^T - ||c||^2/2); row term ||x||^2 dropped.
The cross term is a single float32r matmul pass: the PE array truncates
fp32 inputs to fp22 (13 mantissa bits), which keeps the argmax intact to
~25/65536 near-tie flips on hardware, and with a moving free dim of 512
the fp32r matmul streams at the full one-row/cycle bf16 rate. The
||c||^2/2 bias is precomputed on the host, broadcast to all partitions,
and subtracted on the Vector engine; argmax uses the DVE max/max_index
ops.

The kernel is paced end-to-end by the DVE epilogue (sub+max+max_index =
3446ns/tile vs the PE's 3413ns/tile), so total = (time tile 0's psum is
ready) + 64 x 3446ns + drain. Hence the geometric X block ramp: small
blocks first so tile 0 isn't gated on a full 4MB block transfer
serialized behind the 4MB C transfer, and warmup matmuls to hold the PE
p-state up while the first DMAs land.
"""
import numpy as np
import concourse.bacc as bacc
import concourse.mybir as mybir
from concourse.tile import TileContext
from concourse.bass_utils import run_bass_kernel_spmd

B, F, K = 65536, 1024, 1024
NCORES = 8
BL = B // NCORES          # rows per core
P = 128
FCH = F // P              # 8 contraction chunks
NH = 512                  # psum half (max fp32 moving operand / bank)
# X DMA block sizes (columns): small blocks first so tile 0 isn't gated on a
# full 4MB block transfer behind the 4MB C transfer — the kernel is paced by
# the DVE epilogue from tile 0 onward, so startup latency is pure total time.
BLOCKS = [256, 256, 512, 512] + [1024] * 6 + [512]
NWARM = 8                 # p-state warmup matmuls
DT = mybir.dt.float32r

_NC_CACHE = {}


def _build(bl):
    assert sum(BLOCKS) == bl
    nb = bl // P
    nc = bacc.Bacc("TRN2", target_bir_lowering=False)
    x = nc.dram_tensor("x", [F, bl], DT, kind="ExternalInput")
    c = nc.dram_tensor("c", [F, K], DT, kind="ExternalInput")
    c2h = nc.dram_tensor("c2h", [K], mybir.dt.float32, kind="ExternalInput")
    out = nc.dram_tensor("out", [nb, P, 1], mybir.dt.uint32, kind="ExternalOutput")
    om = nc.dram_tensor("om", [nb, P, NH], mybir.dt.float32,
                        kind="ExternalOutput")

    with TileContext(nc) as tc:
        with (
            tc.tile_pool(name="cres", bufs=1) as cres,
            tc.tile_pool(name="xp", bufs=2) as xp,
            tc.tile_pool(name="work", bufs=3) as work,
            tc.tile_pool(name="psp", bufs=4, space="PSUM") as psp,
        ):
            # Warmup: dummy matmuls into a discarded psum tile eat the PE
            # low/mid p-state ramp while the first input DMAs are in flight,
            # so real matmuls start at full clock.
            dumt = work.tile([P, NH], DT, tag="warm")
            nc.vector.memzero(dumt)
            dps = psp.tile([P, K], mybir.dt.float32, tag="ps")
            for w in range(NWARM):
                nc.tensor.matmul(dps[:, 0:NH], dumt[:, 0:P], dumt,
                                 start=(w == 0), stop=(w == NWARM - 1))

            def load_chunk(c0, ncols, f):
                t = xp.tile([P, ncols], DT, tag=f"x{f}", name=f"xt{f}")
                nc.sync.dma_start(t, x[f * P:(f + 1) * P, c0:c0 + ncols])
                return t

            # Interleave resident-C chunk loads with block-0 X chunk loads so
            # the f-th matmul of tile 0 only waits on the f-th pair; the c2
            # broadcast (needed by the first sub, several tiles later) rides
            # along early.
            c_sb = []
            blk0 = []
            c2b = cres.tile([P, K], mybir.dt.float32)
            for f in range(FCH):
                t_c = cres.tile([P, K], DT, tag=f"c{f}")
                nc.sync.dma_start(t_c, c[f * P:(f + 1) * P, :])
                c_sb.append(t_c)
                blk0.append(load_chunk(0, BLOCKS[0], f))
            # c2b rides the serial DMA stream AFTER block 0 — it's not needed
            # until the first sub (~20us), and placing it earlier delays the
            # tile-0 inputs that gate the whole pipeline.
            nc.sync.dma_start(c2b, c2h[None, :].to_broadcast([P, K]))

            t = 0
            c0 = 0
            for bi, ncols in enumerate(BLOCKS):
                x_t = blk0 if bi == 0 else [load_chunk(c0, ncols, f)
                                            for f in range(FCH)]
                for i in range(ncols // P):
                    ps = psp.tile([P, K], mybir.dt.float32, tag="ps")
                    for f in range(FCH):
                        first = f == 0
                        last = f == FCH - 1
                        w = x_t[f][:, i * P:(i + 1) * P]
                        nc.tensor.matmul(ps[:, 0:NH], w, c_sb[f][:, 0:NH],
                                         start=first, stop=last)
                        nc.tensor.matmul(ps[:, NH:K], w, c_sb[f][:, NH:K],
                                         start=first, stop=last)
                    a_sb = work.tile([P, K], mybir.dt.float32, tag="a")
                    nc.vector.tensor_sub(a_sb, ps, c2b)
                    # DVE folds the 1024 scores to 512 so max/max_index scan
                    # half the data (DVE 2.97us/tile, under the PE's 3.41 —
                    # the PE becomes the pacer). The which-half bit needed to
                    # decode the folded index is computed on the otherwise
                    # idle Pool engine and shipped to the host, so nothing
                    # feeds back into the DVE chain.
                    m = work.tile([P, NH], mybir.dt.float32, tag="m")
                    nc.vector.tensor_max(m, a_sb[:, 0:NH], a_sb[:, NH:K])
                    d = work.tile([P, NH], mybir.dt.float32, tag="d")
                    nc.gpsimd.tensor_sub(d, a_sb[:, 0:NH], a_sb[:, NH:K])
                    mx = work.tile([P, 8], mybir.dt.float32, tag="mx")
                    nc.vector.max(out=mx, in_=m)
                    ix = work.tile([P, 8], mybir.dt.uint32, tag="ix")
                    nc.vector.max_index(ix, mx, m)
                    nc.sync.dma_start(out[t], ix[:, 0:1])
                    nc.scalar.dma_start(om[t], d)
                    t += 1
                c0 += ncols
    nc.finalize()
    return nc


def _get_nc(bl):
    if bl not in _NC_CACHE:
        _NC_CACHE[bl] = _build(bl)
    return _NC_CACHE[bl]


def _prep_in_maps(X, C):
    X = np.ascontiguousarray(np.asarray(X, dtype=np.float32))
    C = np.ascontiguousarray(np.asarray(C, dtype=np.float32))
    assert X.shape == (B, F) and C.shape == (K, F)
    xt = np.ascontiguousarray(X.T)
    ct = np.ascontiguousarray(C.T)
    c2h = (0.5 * np.sum(C.astype(np.float64) ** 2, axis=1)).astype(np.float32)
    in_maps = []
    for cid in range(NCORES):
        sl = slice(cid * BL, (cid + 1) * BL)
        in_maps.append({
            "x": np.ascontiguousarray(xt[:, sl]),
            "c": ct,
            "c2h": c2h,
        })
    return in_maps


def kernel(X, centroids):
    nc = _get_nc(BL)
    in_maps = _prep_in_maps(X, centroids)
    res = run_bass_kernel_spmd(nc, in_maps, core_ids=list(range(NCORES)))
    outs = []
    for r in res.results:
        j = r["out"].reshape(-1).astype(np.int64)          # folded index
        dm = r["om"].reshape(-1, NH)                       # a_lo - a_hi rows
        d = np.take_along_axis(dm, j[:, None], axis=1).reshape(-1)
        outs.append(np.where(d >= 0, j, j + NH))           # decode true k
    return np.concatenate(outs).astype(np.int32)


# revision 24
# speedup vs baseline: 1.0312x; 1.0070x over previous
"""KMeans assignment (vq_codebook) Trainium2 kernel.

argmin_k ||x_b - c_k||^2 for X[65536,1024], C[1024,1024], 8 NeuronCores,
data-parallel over the batch (8192 rows/core), centroids replicated.

Math: argmin_k d2 = argmax_k (X@C^T - ||c||^2/2); row term ||x||^2 dropped.
The cross term is a single float32r matmul pass: the PE array truncates
fp32 inputs to fp22 (13 mantissa bits), which keeps the argmax intact to
~25/65536 near-tie flips on hardware, and with a moving free dim of 512
the fp32r matmul streams at the full one-row/cycle bf16 rate. The
||c||^2/2 bias is precomputed on the host, broadcast to all partitions,
and subtracted on the Vector engine; argmax uses the DVE max/max_index
ops.

The kernel is paced end-to-end by the DVE epilogue (sub+max+max_index =
3446ns/tile vs the PE's 3413ns/tile), so total = (time tile 0's psum is
ready) + 64 x 3446ns + drain. Hence the geometric X block ramp: small
blocks first so tile 0 isn't gated on a full 4MB block transfer
serialized behind the 4MB C transfer, and warmup matmuls to hold the PE
p-state up while the first DMAs land.
"""
import numpy as np
import concourse.bacc as bacc
import concourse.mybir as mybir
from concourse.tile import TileContext
from concourse.bass_utils import run_bass_kernel_spmd

B, F, K = 65536, 1024, 1024
NCORES = 8
BL = B // NCORES          # rows per core
P = 128
FCH = F // P              # 8 contraction chunks
NH = 512                  # psum half (max fp32 moving operand / bank)
# X DMA block sizes (columns): small blocks first so tile 0 isn't gated on a
# full 4MB block transfer behind the 4MB C transfer — the kernel is paced by
# the DVE epilogue from tile 0 onward, so startup latency is pure total time.
BLOCKS = [256, 256, 512, 512] + [1024] * 6 + [512]
NWARM = 8                 # p-state warmup matmuls
DT = mybir.dt.float16

_NC_CACHE = {}


def _build(bl):
    assert sum(BLOCKS) == bl
    nb = bl // P
    nc = bacc.Bacc("TRN2", target_bir_lowering=False)
    x = nc.dram_tensor("x", [F, bl], DT, kind="ExternalInput")
    c = nc.dram_tensor("c", [F, K], mybir.dt.float16,
                        kind="ExternalInput")
    c2h = nc.dram_tensor("c2h", [K], mybir.dt.float32, kind="ExternalInput")
    out = nc.dram_tensor("out", [nb, P, 1], mybir.dt.uint32, kind="ExternalOutput")
    om = nc.dram_tensor("om", [nb, P, NH], mybir.dt.float32,
                        kind="ExternalOutput")

    with TileContext(nc) as tc:
        with (
            tc.tile_pool(name="cres", bufs=1) as cres,
            tc.tile_pool(name="xp", bufs=2) as xp,
            tc.tile_pool(name="work", bufs=3) as work,
            tc.tile_pool(name="psp", bufs=4, space="PSUM") as psp,
        ):
            # Warmup: dummy matmuls into a discarded psum tile eat the PE
            # low/mid p-state ramp while the first input DMAs are in flight,
            # so real matmuls start at full clock.
            dumt = work.tile([P, NH], DT, tag="warm")
            nc.vector.memzero(dumt)
            dps = psp.tile([P, K], mybir.dt.float32, tag="ps")
            for w in range(NWARM):
                nc.tensor.matmul(dps[:, 0:NH], dumt[:, 0:P], dumt,
                                 start=(w == 0), stop=(w == NWARM - 1))

            def load_chunk(c0, ncols, f):
                t = xp.tile([P, ncols], DT, tag=f"x{f}", name=f"xt{f}")
                nc.sync.dma_start(t, x[f * P:(f + 1) * P, c0:c0 + ncols])
                return t

            # Interleave resident-C chunk loads with block-0 X chunk loads so
            # the f-th matmul of tile 0 only waits on the f-th pair; the c2
            # broadcast (needed by the first sub, several tiles later) rides
            # along early.
            c_sb = []
            blk0 = []
            c2b = cres.tile([P, K], mybir.dt.float32)
            for f in range(FCH):
                t_c = cres.tile([P, K], mybir.dt.float16, tag=f"c{f}")
                nc.sync.dma_start(t_c, c[f * P:(f + 1) * P, :])
                c_sb.append(t_c)
                blk0.append(load_chunk(0, BLOCKS[0], f))
            # c2b rides the serial DMA stream AFTER block 0 — it's not needed
            # until the first sub (~20us), and placing it earlier delays the
            # tile-0 inputs that gate the whole pipeline.
            nc.sync.dma_start(c2b, c2h[None, :].to_broadcast([P, K]))

            t = 0
            c0 = 0
            for bi, ncols in enumerate(BLOCKS):
                x_t = blk0 if bi == 0 else [load_chunk(c0, ncols, f)
                                            for f in range(FCH)]
                for i in range(ncols // P):
                    ps = psp.tile([P, K], mybir.dt.float32, tag="ps")
                    for f in range(FCH):
                        first = f == 0
                        last = f == FCH - 1
                        w = x_t[f][:, i * P:(i + 1) * P]
                        nc.tensor.matmul(ps[:, 0:NH], w, c_sb[f][:, 0:NH],
                                         start=first, stop=last)
                        nc.tensor.matmul(ps[:, NH:K], w, c_sb[f][:, NH:K],
                                         start=first, stop=last)
                    a_sb = work.tile([P, K], mybir.dt.float32, tag="a")
                    nc.vector.tensor_sub(a_sb, ps, c2b)
                    # DVE folds the 1024 scores to 512 so max/max_index scan
                    # half the data (DVE 2.97us/tile, under the PE's 3.41 —
                    # the PE becomes the pacer). The which-half bit needed to
                    # decode the folded index is computed on the otherwise
                    # idle Pool engine and shipped to the host, so nothing
                    # feeds back into the DVE chain.
                    m = work.tile([P, NH], mybir.dt.float32, tag="m")
                    nc.vector.tensor_max(m, a_sb[:, 0:NH], a_sb[:, NH:K])
                    d = work.tile([P, NH], mybir.dt.float32, tag="d")
                    nc.gpsimd.tensor_sub(d, a_sb[:, 0:NH], a_sb[:, NH:K])
                    mx = work.tile([P, 8], mybir.dt.float32, tag="mx")
                    nc.vector.max(out=mx, in_=m)
                    ix = work.tile([P, 8], mybir.dt.uint32, tag="ix")
                    nc.vector.max_index(ix, mx, m)
                    nc.sync.dma_start(out[t], ix[:, 0:1])
                    nc.scalar.dma_start(om[t], d)
                    t += 1
                c0 += ncols
    nc.finalize()
    return nc


def _get_nc(bl):
    if bl not in _NC_CACHE:
        _NC_CACHE[bl] = _build(bl)
    return _NC_CACHE[bl]


def _prep_in_maps(X, C):
    X = np.ascontiguousarray(np.asarray(X, dtype=np.float32))
    C = np.ascontiguousarray(np.asarray(C, dtype=np.float32))
    assert X.shape == (B, F) and C.shape == (K, F)
    xt = np.ascontiguousarray(X.T.astype(np.float16))
    ct = np.ascontiguousarray(C.T.astype(np.float16))
    c2h = (0.5 * np.sum(C.astype(np.float64) ** 2, axis=1)).astype(np.float32)
    in_maps = []
    for cid in range(NCORES):
        sl = slice(cid * BL, (cid + 1) * BL)
        in_maps.append({
            "x": np.ascontiguousarray(xt[:, sl]),
            "c": ct,
            "c2h": c2h,
        })
    return in_maps


def kernel(X, centroids):
    nc = _get_nc(BL)
    in_maps = _prep_in_maps(X, centroids)
    res = run_bass_kernel_spmd(nc, in_maps, core_ids=list(range(NCORES)))
    outs = []
    for r in res.results:
        j = r["out"].reshape(-1).astype(np.int64)          # folded index
        dm = r["om"].reshape(-1, NH)                       # a_lo - a_hi rows
        d = np.take_along_axis(dm, j[:, None], axis=1).reshape(-1)
        outs.append(np.where(d >= 0, j, j + NH))           # decode true k
    return np.concatenate(outs).astype(np.int32)


# revision 25
# speedup vs baseline: 1.0575x; 1.0255x over previous
"""KMeans assignment (vq_codebook) Trainium2 kernel.

argmin_k ||x_b - c_k||^2 for X[65536,1024], C[1024,1024], 8 NeuronCores,
data-parallel over the batch (8192 rows/core), centroids replicated.

Math: argmin_k d2 = argmax_k (X@C^T - ||c||^2/2); row term ||x||^2 dropped.
The cross term is a single float32r matmul pass: the PE array truncates
fp32 inputs to fp22 (13 mantissa bits), which keeps the argmax intact to
~25/65536 near-tie flips on hardware, and with a moving free dim of 512
the fp32r matmul streams at the full one-row/cycle bf16 rate. The
||c||^2/2 bias is precomputed on the host, broadcast to all partitions,
and subtracted on the Vector engine; argmax uses the DVE max/max_index
ops.

The kernel is paced end-to-end by the DVE epilogue (sub+max+max_index =
3446ns/tile vs the PE's 3413ns/tile), so total = (time tile 0's psum is
ready) + 64 x 3446ns + drain. Hence the geometric X block ramp: small
blocks first so tile 0 isn't gated on a full 4MB block transfer
serialized behind the 4MB C transfer, and warmup matmuls to hold the PE
p-state up while the first DMAs land.
"""
import numpy as np
import concourse.bacc as bacc
import concourse.mybir as mybir
from concourse.tile import TileContext
from concourse.bass_utils import run_bass_kernel_spmd

B, F, K = 65536, 1024, 1024
NCORES = 8
BL = B // NCORES          # rows per core
P = 128
FCH = F // P              # 8 contraction chunks
NH = 512                  # psum half (max fp32 moving operand / bank)
# X DMA block sizes (columns): small blocks first so tile 0 isn't gated on a
# full 4MB block transfer behind the 4MB C transfer — the kernel is paced by
# the DVE epilogue from tile 0 onward, so startup latency is pure total time.
BLOCKS = [1024] * 8
NWARM = 8                 # p-state warmup matmuls
DT = mybir.dt.float16

_NC_CACHE = {}


def _build(bl):
    assert sum(BLOCKS) == bl
    nb = bl // P
    nc = bacc.Bacc("TRN2", target_bir_lowering=False)
    x = nc.dram_tensor("x", [F, bl], DT, kind="ExternalInput")
    c = nc.dram_tensor("c", [F, K], mybir.dt.float16,
                        kind="ExternalInput")
    c2h = nc.dram_tensor("c2h", [K], mybir.dt.float32, kind="ExternalInput")
    out = nc.dram_tensor("out", [nb, P, 1], mybir.dt.uint32, kind="ExternalOutput")
    om = nc.dram_tensor("om", [nb, P, NH], mybir.dt.float32,
                        kind="ExternalOutput")

    with TileContext(nc) as tc:
        with (
            tc.tile_pool(name="cres", bufs=1) as cres,
            tc.tile_pool(name="xp", bufs=2) as xp,
            tc.tile_pool(name="work", bufs=3) as work,
            tc.tile_pool(name="psp", bufs=4, space="PSUM") as psp,
        ):
            # Warmup: dummy matmuls into a discarded psum tile eat the PE
            # low/mid p-state ramp while the first input DMAs are in flight,
            # so real matmuls start at full clock.
            dumt = work.tile([P, NH], DT, tag="warm")
            nc.vector.memzero(dumt)
            dps = psp.tile([P, K], mybir.dt.float32, tag="ps")
            for w in range(NWARM):
                nc.tensor.matmul(dps[:, 0:NH], dumt[:, 0:P], dumt,
                                 start=(w == 0), stop=(w == NWARM - 1))

            def load_chunk(c0, ncols, f):
                t = xp.tile([P, ncols], DT, tag=f"x{f}", name=f"xt{f}")
                nc.sync.dma_start(t, x[f * P:(f + 1) * P, c0:c0 + ncols])
                return t

            # Interleave resident-C chunk loads with block-0 X chunk loads so
            # the f-th matmul of tile 0 only waits on the f-th pair; the c2
            # broadcast (needed by the first sub, several tiles later) rides
            # along early.
            c_sb = []
            blk0 = []
            c2b = cres.tile([P, K], mybir.dt.float32)
            for f in range(FCH):
                t_c = cres.tile([P, K], mybir.dt.float16, tag=f"c{f}")
                nc.sync.dma_start(t_c, c[f * P:(f + 1) * P, :])
                c_sb.append(t_c)
                blk0.append(load_chunk(0, BLOCKS[0], f))
            # c2b rides the serial DMA stream AFTER block 0 — it's not needed
            # until the first sub (~20us), and placing it earlier delays the
            # tile-0 inputs that gate the whole pipeline.
            nc.sync.dma_start(c2b, c2h[None, :].to_broadcast([P, K]))

            t = 0
            c0 = 0
            for bi, ncols in enumerate(BLOCKS):
                x_t = blk0 if bi == 0 else [load_chunk(c0, ncols, f)
                                            for f in range(FCH)]
                for i in range(ncols // P):
                    ps = psp.tile([P, K], mybir.dt.float32, tag="ps")
                    for f in range(FCH):
                        first = f == 0
                        last = f == FCH - 1
                        w = x_t[f][:, i * P:(i + 1) * P]
                        nc.tensor.matmul(ps[:, 0:NH], w, c_sb[f][:, 0:NH],
                                         start=first, stop=last)
                        nc.tensor.matmul(ps[:, NH:K], w, c_sb[f][:, NH:K],
                                         start=first, stop=last)
                    a_sb = work.tile([P, K], mybir.dt.float32, tag="a")
                    nc.vector.tensor_sub(a_sb, ps, c2b)
                    # DVE folds the 1024 scores to 512 so max/max_index scan
                    # half the data (DVE 2.97us/tile, under the PE's 3.41 —
                    # the PE becomes the pacer). The which-half bit needed to
                    # decode the folded index is computed on the otherwise
                    # idle Pool engine and shipped to the host, so nothing
                    # feeds back into the DVE chain.
                    m = work.tile([P, NH], mybir.dt.float32, tag="m")
                    nc.vector.tensor_max(m, a_sb[:, 0:NH], a_sb[:, NH:K])
                    d = work.tile([P, NH], mybir.dt.float32, tag="d")
                    nc.gpsimd.tensor_sub(d, a_sb[:, 0:NH], a_sb[:, NH:K])
                    mx = work.tile([P, 8], mybir.dt.float32, tag="mx")
                    nc.vector.max(out=mx, in_=m)
                    ix = work.tile([P, 8], mybir.dt.uint32, tag="ix")
                    nc.vector.max_index(ix, mx, m)
                    nc.sync.dma_start(out[t], ix[:, 0:1])
                    nc.scalar.dma_start(om[t], d)
                    t += 1
                c0 += ncols
    nc.finalize()
    return nc


def _get_nc(bl):
    if bl not in _NC_CACHE:
        _NC_CACHE[bl] = _build(bl)
    return _NC_CACHE[bl]


def _prep_in_maps(X, C):
    X = np.ascontiguousarray(np.asarray(X, dtype=np.float32))
    C = np.ascontiguousarray(np.asarray(C, dtype=np.float32))
    assert X.shape == (B, F) and C.shape == (K, F)
    xt = np.ascontiguousarray(X.T.astype(np.float16))
    ct = np.ascontiguousarray(C.T.astype(np.float16))
    c2h = (0.5 * np.sum(C.astype(np.float64) ** 2, axis=1)).astype(np.float32)
    in_maps = []
    for cid in range(NCORES):
        sl = slice(cid * BL, (cid + 1) * BL)
        in_maps.append({
            "x": np.ascontiguousarray(xt[:, sl]),
            "c": ct,
            "c2h": c2h,
        })
    return in_maps


def kernel(X, centroids):
    nc = _get_nc(BL)
    in_maps = _prep_in_maps(X, centroids)
    res = run_bass_kernel_spmd(nc, in_maps, core_ids=list(range(NCORES)))
    outs = []
    for r in res.results:
        j = r["out"].reshape(-1).astype(np.int64)          # folded index
        dm = r["om"].reshape(-1, NH)                       # a_lo - a_hi rows
        d = np.take_along_axis(dm, j[:, None], axis=1).reshape(-1)
        outs.append(np.where(d >= 0, j, j + NH))           # decode true k
    return np.concatenate(outs).astype(np.int32)


# revision 26
# speedup vs baseline: 1.0588x; 1.0012x over previous
"""KMeans assignment (vq_codebook) Trainium2 kernel.

argmin_k ||x_b - c_k||^2 for X[65536,1024], C[1024,1024], 8 NeuronCores,
data-parallel over the batch (8192 rows/core), centroids replicated.

Math: argmin_k d2 = argmax_k (X@C^T - ||c||^2/2); row term ||x||^2 dropped.
The cross term is a single float32r matmul pass: the PE array truncates
fp32 inputs to fp22 (13 mantissa bits), which keeps the argmax intact to
~25/65536 near-tie flips on hardware, and with a moving free dim of 512
the fp32r matmul streams at the full one-row/cycle bf16 rate. The
||c||^2/2 bias is precomputed on the host, broadcast to all partitions,
and subtracted on the Vector engine; argmax uses the DVE max/max_index
ops.

The kernel is paced end-to-end by the DVE epilogue (sub+max+max_index =
3446ns/tile vs the PE's 3413ns/tile), so total = (time tile 0's psum is
ready) + 64 x 3446ns + drain. Hence the geometric X block ramp: small
blocks first so tile 0 isn't gated on a full 4MB block transfer
serialized behind the 4MB C transfer, and warmup matmuls to hold the PE
p-state up while the first DMAs land.
"""
import numpy as np
import concourse.bacc as bacc
import concourse.mybir as mybir
from concourse.tile import TileContext
from concourse.bass_utils import run_bass_kernel_spmd

B, F, K = 65536, 1024, 1024
NCORES = 8
BL = B // NCORES          # rows per core
P = 128
FCH = F // P              # 8 contraction chunks
NH = 512                  # psum half (max fp32 moving operand / bank)
# X DMA block sizes (columns): small blocks first so tile 0 isn't gated on a
# full 4MB block transfer behind the 4MB C transfer — the kernel is paced by
# the DVE epilogue from tile 0 onward, so startup latency is pure total time.
BLOCKS = [1024] * 8
NWARM = 24                # p-state warmup matmuls
DT = mybir.dt.float16

_NC_CACHE = {}


def _build(bl):
    assert sum(BLOCKS) == bl
    nb = bl // P
    nc = bacc.Bacc("TRN2", target_bir_lowering=False)
    x = nc.dram_tensor("x", [F, bl], DT, kind="ExternalInput")
    c = nc.dram_tensor("c", [F, K], mybir.dt.float16,
                        kind="ExternalInput")
    c2h = nc.dram_tensor("c2h", [K], mybir.dt.float32, kind="ExternalInput")
    out = nc.dram_tensor("out", [nb, P, 1], mybir.dt.uint32, kind="ExternalOutput")
    om = nc.dram_tensor("om", [nb, P, NH], mybir.dt.float32,
                        kind="ExternalOutput")

    with TileContext(nc) as tc:
        with (
            tc.tile_pool(name="cres", bufs=1) as cres,
            tc.tile_pool(name="xp", bufs=2) as xp,
            tc.tile_pool(name="work", bufs=3) as work,
            tc.tile_pool(name="psp", bufs=4, space="PSUM") as psp,
        ):
            # Warmup: dummy matmuls into a discarded psum tile eat the PE
            # low/mid p-state ramp while the first input DMAs are in flight,
            # so real matmuls start at full clock.
            dumt = work.tile([P, P], DT, tag="warm")
            nc.vector.memzero(dumt)
            dps = psp.tile([P, K], mybir.dt.float32, tag="ps")
            for w in range(NWARM):
                nc.tensor.matmul(dps[:, 0:P], dumt, dumt,
                                 start=(w == 0), stop=(w == NWARM - 1))

            def load_chunk(c0, ncols, f):
                t = xp.tile([P, ncols], DT, tag=f"x{f}", name=f"xt{f}")
                nc.sync.dma_start(t, x[f * P:(f + 1) * P, c0:c0 + ncols])
                return t

            # Interleave resident-C chunk loads with block-0 X chunk loads so
            # the f-th matmul of tile 0 only waits on the f-th pair; the c2
            # broadcast (needed by the first sub, several tiles later) rides
            # along early.
            c_sb = []
            blk0 = []
            c2b = cres.tile([P, K], mybir.dt.float32)
            for f in range(FCH):
                t_c = cres.tile([P, K], mybir.dt.float16, tag=f"c{f}")
                nc.sync.dma_start(t_c, c[f * P:(f + 1) * P, :])
                c_sb.append(t_c)
                blk0.append(load_chunk(0, BLOCKS[0], f))
            # c2b rides the serial DMA stream AFTER block 0 — it's not needed
            # until the first sub (~20us), and placing it earlier delays the
            # tile-0 inputs that gate the whole pipeline.
            nc.sync.dma_start(c2b, c2h[None, :].to_broadcast([P, K]))

            t = 0
            c0 = 0
            for bi, ncols in enumerate(BLOCKS):
                x_t = blk0 if bi == 0 else [load_chunk(c0, ncols, f)
                                            for f in range(FCH)]
                for i in range(ncols // P):
                    ps = psp.tile([P, K], mybir.dt.float32, tag="ps")
                    for f in range(FCH):
                        first = f == 0
                        last = f == FCH - 1
                        w = x_t[f][:, i * P:(i + 1) * P]
                        nc.tensor.matmul(ps[:, 0:NH], w, c_sb[f][:, 0:NH],
                                         start=first, stop=last)
                        nc.tensor.matmul(ps[:, NH:K], w, c_sb[f][:, NH:K],
                                         start=first, stop=last)
                    a_sb = work.tile([P, K], mybir.dt.float32, tag="a")
                    nc.vector.tensor_sub(a_sb, ps, c2b)
                    # DVE folds the 1024 scores to 512 so max/max_index scan
                    # half the data (DVE 2.97us/tile, under the PE's 3.41 —
                    # the PE becomes the pacer). The which-half bit needed to
                    # decode the folded index is computed on the otherwise
                    # idle Pool engine and shipped to the host, so nothing
                    # feeds back into the DVE chain.
                    m = work.tile([P, NH], mybir.dt.float32, tag="m")
                    nc.vector.tensor_max(m, a_sb[:, 0:NH], a_sb[:, NH:K])
                    d = work.tile([P, NH], mybir.dt.float32, tag="d")
                    nc.gpsimd.tensor_sub(d, a_sb[:, 0:NH], a_sb[:, NH:K])
                    mx = work.tile([P, 8], mybir.dt.float32, tag="mx")
                    nc.vector.max(out=mx, in_=m)
                    ix = work.tile([P, 8], mybir.dt.uint32, tag="ix")
                    nc.vector.max_index(ix, mx, m)
                    nc.sync.dma_start(out[t], ix[:, 0:1])
                    nc.scalar.dma_start(om[t], d)
                    t += 1
                c0 += ncols
    nc.finalize()
    return nc


def _get_nc(bl):
    if bl not in _NC_CACHE:
        _NC_CACHE[bl] = _build(bl)
    return _NC_CACHE[bl]


def _prep_in_maps(X, C):
    X = np.ascontiguousarray(np.asarray(X, dtype=np.float32))
    C = np.ascontiguousarray(np.asarray(C, dtype=np.float32))
    assert X.shape == (B, F) and C.shape == (K, F)
    xt = np.ascontiguousarray(X.T.astype(np.float16))
    ct = np.ascontiguousarray(C.T.astype(np.float16))
    c2h = (0.5 * np.sum(C.astype(np.float64) ** 2, axis=1)).astype(np.float32)
    in_maps = []
    for cid in range(NCORES):
        sl = slice(cid * BL, (cid + 1) * BL)
        in_maps.append({
            "x": np.ascontiguousarray(xt[:, sl]),
            "c": ct,
            "c2h": c2h,
        })
    return in_maps


def kernel(X, centroids):
    nc = _get_nc(BL)
    in_maps = _prep_in_maps(X, centroids)
    res = run_bass_kernel_spmd(nc, in_maps, core_ids=list(range(NCORES)))
    outs = []
    for r in res.results:
        j = r["out"].reshape(-1).astype(np.int64)          # folded index
        dm = r["om"].reshape(-1, NH)                       # a_lo - a_hi rows
        d = np.take_along_axis(dm, j[:, None], axis=1).reshape(-1)
        outs.append(np.where(d >= 0, j, j + NH))           # decode true k
    return np.concatenate(outs).astype(np.int32)
